# revision 8
# baseline (speedup 1.0000x reference)
"""Conv1dSubsamplingShrink on 8 Trainium2 NeuronCores.

Data-parallel over the batch (32 seqs -> 4 per core). Per core:
  x (3000, 4, 80) -> conv1(80->1024, K=5, s=2) -> GLU -> (512, 1500)
                  -> conv2(512->1024, K=5, s=2) -> GLU -> (750, 512)

conv1: weights stationary [80, 128co], x moving with stride-2 free-dim APs,
       accumulated over 5 taps in PSUM -> out [co_block, t].
conv2: y1 slices stationary [128ci, t_block] (stride-2), w2 moving
       [128ci, 512co], 20 accumulating matmuls -> out [t_block, co] which is
       already the output layout (no final transpose needed).
x is transposed to channels-on-partitions once via PE-transpose.
"""

import os
import sys

import numpy as np

if "/opt/trn_rl_repo" not in sys.path:
    sys.path.insert(0, "/opt/trn_rl_repo")

T_IN, BSZ, D_IN = 3000, 32, 80
MID, OUT, K = 1024, 512, 5
NCORES = 8
BPC = BSZ // NCORES          # sequences per core
T1, T2 = 1500, 750           # time dims after conv1 / conv2
TP = T_IN + 4                # padded input time (pad=2 each side)
T1P = T1 + 4                 # padded y1 time
NBLK = (T_IN + 127) // 128   # 24 t-blocks for the input transpose
CH1 = 500                    # conv1 output chunk (<=512 PSUM bank)

MM_DT = os.environ.get("CONV_MM_DT", "f32r")  # f32 | f32r | fp16 | bf16

LAST_RESULT = None
_prog_cache = {}


def _build(mm_dt, use_b1, use_b2):
    from contextlib import ExitStack

    import concourse.tile as tile
    import concourse.mybir as mybir
    from concourse import bacc
    from concourse.alu_op_type import AluOpType
    from concourse.masks import make_identity

    f32 = mybir.dt.float32
    if mm_dt == "fp16":
        dt_st = mybir.dt.float16
    elif mm_dt == "bf16":
        dt_st = mybir.dt.bfloat16
    elif mm_dt == "f32r":
        dt_st = mybir.dt.float32r
    else:
        dt_st = f32
    cast16 = dt_st in (mybir.dt.float16, mybir.dt.bfloat16)
    # dtype of the transpose path (PE transpose runs in plain f32 for the
    # 4-byte configs; the rounding to f32r happens on the PSUM->SBUF copy)
    dt_tp = dt_st if cast16 else f32

    def mm(ap):
        return ap

    def zap(ap):
        # memset can't encode an f32r immediate; zero bits are identical in f32
        return ap.bitcast(f32) if dt_st == mybir.dt.float32r else ap

    SIG = mybir.ActivationFunctionType.Sigmoid

    nc = bacc.Bacc("TRN2", target_bir_lowering=False, debug=False,
                   num_devices=NCORES)
    x_d = nc.dram_tensor("x", (T_IN, BPC, D_IN), f32, kind="ExternalInput").ap()
    w1_d = nc.dram_tensor("w1p", (D_IN, K * MID), dt_st, kind="ExternalInput").ap()
    w2_d = nc.dram_tensor("w2p", (128, 4 * K * MID), dt_st, kind="ExternalInput").ap()
    b1_d = nc.dram_tensor("b1p", (128, 8), f32, kind="ExternalInput").ap()
    b2_d = nc.dram_tensor("b2m", (128, MID), f32, kind="ExternalInput").ap()
    y_d = nc.dram_tensor("y", (T2, BPC, OUT), f32, kind="ExternalOutput").ap()

    with tile.TileContext(nc) as tc, ExitStack() as ctx:
        const = ctx.enter_context(tc.tile_pool(name="const", bufs=1))
        w2_sb = const.tile([128, 4 * K * MID], dt_st)
        nc.sync.dma_start(w2_sb[:], w2_d[:])
        w1_sb = const.tile([128, K * MID], dt_st)
        nc.sync.dma_start(w1_sb[:D_IN], w1_d[:])
        b1_sb = const.tile([128, 8], f32)
        nc.sync.dma_start(b1_sb[:], b1_d[:])
        b2_sb = const.tile([128, MID], f32)
        nc.sync.dma_start(b2_sb[:], b2_d[:])
        ident = const.tile([128, 128], dt_tp)
        make_identity(nc, ident[:])

        xnat_pool = ctx.enter_context(tc.tile_pool(name="xnat", bufs=3))
        xc_pool = ctx.enter_context(tc.tile_pool(name="xc", bufs=4))
        tp_ps = ctx.enter_context(tc.tile_pool(name="tp_ps", bufs=2, space="PSUM"))
        x_pool = ctx.enter_context(tc.tile_pool(name="xsb", bufs=2))
        y1_pool = ctx.enter_context(tc.tile_pool(name="y1", bufs=2))
        sig_pool = ctx.enter_context(tc.tile_pool(name="sig", bufs=2))
        ps_pool = ctx.enter_context(tc.tile_pool(name="ps", bufs=3, space="PSUM"))
        y2_pool = ctx.enter_context(tc.tile_pool(name="y2", bufs=2))

        for s in range(BPC):
            nfull = T_IN // 128                  # 23 full blocks
            tail = T_IN - nfull * 128            # 56

            # ---- transpose to x_sb [d partitions, t] with 2-col zero pads
            x_sb = x_pool.tile([128, TP], dt_st)
            nc.vector.memset(zap(x_sb[:D_IN, 0:2]), 0.0)
            nc.vector.memset(zap(x_sb[:D_IN, TP - 2: TP]), 0.0)
            for blk in range(NBLK):
                tb = 128 if blk < nfull else tail
                xnb = xnat_pool.tile([128, D_IN], f32)
                nc.sync.dma_start(xnb[:tb], x_d[blk * 128: blk * 128 + tb, s, :])
                src = xnb[:tb]
                if cast16:
                    xc = xc_pool.tile([128, D_IN], dt_tp)
                    nc.vector.tensor_copy(xc[:tb], src)
                    tin = xc[:tb]
                else:
                    tin = src
                pt = tp_ps.tile([128, 128], dt_tp)
                nc.tensor.transpose(pt[:D_IN, :tb], tin, ident[:tb, :tb])
                nc.any.tensor_copy(x_sb[:D_IN, 2 + blk * 128: 2 + blk * 128 + tb],
                                   pt[:D_IN, :tb])

            # ---- conv1 + GLU -> y1_sb [ci_block q partitions, t (padded)]
            y1_sb = y1_pool.tile([128, 4 * T1P], dt_st)
            for q in range(4):
                nc.vector.memset(zap(y1_sb[:, q * T1P: q * T1P + 2]), 0.0)
                nc.vector.memset(zap(y1_sb[:, q * T1P + T1 + 2: (q + 1) * T1P]), 0.0)
            for c in range(T1 // CH1):
                c0 = c * CH1
                for j in range(4):
                    psA = ps_pool.tile([128, 512], mybir.dt.float32)
                    psB = ps_pool.tile([128, 512], mybir.dt.float32)
                    for ps, jj in ((psA, j), (psB, j + 4)):
                        for k in range(K):
                            lhs = w1_sb[:D_IN, k * MID + jj * 128: k * MID + (jj + 1) * 128]
                            rhs = x_sb[:D_IN, k + 2 * c0: k + 2 * c0 + 2 * CH1: 2]
                            nc.tensor.matmul(ps[:, :CH1], mm(lhs), mm(rhs),
                                             start=(k == 0), stop=(k == K - 1))
                    sig = sig_pool.tile([128, 512], mybir.dt.float32)
                    ydst = y1_sb[:, j * T1P + 2 + c0: j * T1P + 2 + c0 + CH1]
                    if use_b1:
                        nc.scalar.activation(sig[:, :CH1], psB[:, :CH1], SIG,
                                             bias=b1_sb[:, j + 4: j + 5])
                        nc.vector.scalar_tensor_tensor(
                            ydst, psA[:, :CH1], b1_sb[:, j: j + 1], sig[:, :CH1],
                            op0=AluOpType.add, op1=AluOpType.mult)
                    else:
                        nc.scalar.activation(sig[:, :CH1], psB[:, :CH1], SIG)
                        nc.vector.tensor_tensor(ydst, psA[:, :CH1], sig[:, :CH1],
                                                op=AluOpType.mult)

            # ---- conv2 + GLU -> y [t, s, co] directly
            for tb in range((T2 + 127) // 128):
                t0 = tb * 128
                M = min(128, T2 - t0)
                psA = ps_pool.tile([128, 512], mybir.dt.float32)
                psB = ps_pool.tile([128, 512], mybir.dt.float32)
                for ps, half in ((psA, 0), (psB, 1)):
                    nmm = 0
                    for q in range(4):
                        for k in range(K):
                            lhs = y1_sb[:, q * T1P + 2 * t0 + k:
                                        q * T1P + 2 * t0 + k + 2 * M: 2]
                            rhs = w2_sb[:, q * K * MID + k * MID + half * 512:
                                        q * K * MID + k * MID + half * 512 + 512]
                            nc.tensor.matmul(ps[:M, :], mm(lhs), mm(rhs),
                                             start=(nmm == 0), stop=(nmm == 19))
                            nmm += 1
                sig2 = sig_pool.tile([128, 512], mybir.dt.float32)
                if use_b2:
                    nc.vector.tensor_add(psB[:M], psB[:M], b2_sb[:M, 512:])
                    nc.vector.tensor_add(psA[:M], psA[:M], b2_sb[:M, :512])
                nc.scalar.activation(sig2[:M], psB[:M], SIG)
                y2t = y2_pool.tile([128, 512], mybir.dt.float32)
                nc.vector.tensor_tensor(y2t[:M], psA[:M], sig2[:M],
                                        op=AluOpType.mult)
                nc.sync.dma_start(y_d[t0: t0 + M, s, :], y2t[:M])

    nc.compile()
    return nc


def _get_program(mm_dt, use_b1, use_b2):
    key = (mm_dt, use_b1, use_b2)
    if key not in _prog_cache:
        _prog_cache[key] = _build(mm_dt, use_b1, use_b2)
    return _prog_cache[key]


def _pack_inputs(src_tokens, w1, b1, w2, b2, mm_dt):
    np_st = {"fp16": np.float16, "bf16": None}.get(mm_dt, np.float32)
    if mm_dt == "bf16":
        import ml_dtypes
        np_st = ml_dtypes.bfloat16
    x = np.ascontiguousarray(src_tokens, dtype=np.float32)
    w1p = np.ascontiguousarray(
        w1.astype(np.float32).transpose(1, 2, 0).reshape(D_IN, K * MID).astype(np_st))
    w2p = np.ascontiguousarray(
        w2.astype(np.float32).transpose(1, 2, 0)          # (512 ci, 5 k, 1024 co)
        .reshape(4, 128, K * MID).transpose(1, 0, 2)      # (128, 4 q, 5120)
        .reshape(128, 4 * K * MID).astype(np_st))
    b1p = np.ascontiguousarray(b1.astype(np.float32).reshape(8, 128).T)
    b2m = np.ascontiguousarray(
        np.broadcast_to(b2.astype(np.float32), (128, MID)))
    return x, w1p, w2p, b1p, b2m


def kernel(src_tokens, src_lengths, w1, b1, w2, b2):
    from concourse import bass_utils

    mm_dt = MM_DT
    x, w1p, w2p, b1p, b2m = _pack_inputs(src_tokens, w1, b1, w2, b2, mm_dt)
    use_b1 = bool(np.any(b1))
    use_b2 = bool(np.any(b2))
    nc = _get_program(mm_dt, use_b1, use_b2)

    in_maps = []
    for c in range(NCORES):
        in_maps.append({
            "x": np.ascontiguousarray(x[:, c * BPC:(c + 1) * BPC, :]),
            "w1p": w1p, "w2p": w2p, "b1p": b1p, "b2m": b2m,
        })
    trace = bool(int(os.environ.get("CONV_TRACE", "0")))
    res = bass_utils.run_bass_kernel_spmd(
        nc, in_maps, core_ids=list(range(NCORES)), trace=trace)
    global LAST_RESULT
    LAST_RESULT = res

    out = np.empty((T2, BSZ, OUT), np.float32)
    for c in range(NCORES):
        out[:, c * BPC:(c + 1) * BPC, :] = res.results[c]["y"]

    lens = np.asarray(src_lengths).astype(np.int64)
    lens = (lens + 1) // 2
    lens = (lens + 1) // 2
    return out, lens.astype(np.int32)
